# revision 11
# baseline (speedup 1.0000x reference)
"""Trainium2 Bass kernel for causal multi-head attention block.

B=128, T=256, C=384, H=6, Dh=64. Data-parallel over batch: 16 batches per
core on 8 NeuronCores. Weights replicated; no collectives.

Per-core dataflow (all feature-major until the PV output):
  XT [384, 4096] bf16 (host pre-transposed)
  QKV^T: per batch, Q^T/K^T feature-major [768, 256], V token-major [256, 384+1]
  S^T[k,q] = K Q^T per head (causal-skipped), mask added via identity-matmul
  P = exp(S^T * 1/8) on ACT (no max subtraction: |S*scale| <= ~3)
  O[q, 65] = P^T.T @ V_aug  (P-stationary) -> rowsum in col 64
  normalize per-partition (q), PE-transpose O -> O^T, proj, +bias, DMA out.
"""

import sys

sys.path.insert(0, "/opt/trn_rl_repo")

import numpy as np
import ml_dtypes

import concourse.bass as bass
import concourse.mybir as mybir
import concourse.tile as tile
from concourse import bacc
from concourse.bass_utils import run_bass_kernel_spmd
from concourse.masks import make_identity

BF16 = mybir.dt.bfloat16
F32 = mybir.dt.float32

N_CORES = 8
B_FULL, T, C = 128, 256, 384
H, DH = 6, 64
BPC = B_FULL // N_CORES  # 16 batches per core
NTOK = BPC * T  # 4096 tokens per core
SCALE = 1.0 / 8.0  # 1/sqrt(64)
NEG = -30000.0  # mask add; exp(NEG*SCALE) == 0 in f32/bf16

_CACHE = {}


def build_kernel():
    nc = bacc.Bacc()
    xT = nc.declare_dram_parameter("xT", [C, NTOK], BF16, isOutput=False)
    wqkvT = nc.declare_dram_parameter("wqkvT", [C, 3 * C], BF16, isOutput=False)
    wprojT = nc.declare_dram_parameter("wprojT", [C, C], BF16, isOutput=False)
    bqkv = nc.declare_dram_parameter("bqkv", [3 * C], F32, isOutput=False)
    bproj = nc.declare_dram_parameter("bproj", [C], F32, isOutput=False)
    out = nc.declare_dram_parameter("out", [NTOK, C], F32, isOutput=True)

    with tile.TileContext(nc) as tc:
        with (
            tc.tile_pool(name="consts", bufs=1) as consts,
            tc.tile_pool(name="xbp", bufs=3) as xbp,
            tc.tile_pool(name="qkp", bufs=2) as qkp,
            tc.tile_pool(name="vp", bufs=2) as vp,
            tc.tile_pool(name="pp", bufs=2) as pp,
            tc.tile_pool(name="op", bufs=3) as op_pool,
            tc.tile_pool(name="otp", bufs=2) as otp,
            tc.tile_pool(name="yp", bufs=3) as yp,
            tc.tile_pool(name="ps_gen", bufs=2, space="PSUM") as ps_gen,
            tc.tile_pool(name="ps_s", bufs=2, space="PSUM") as ps_s,
            tc.tile_pool(name="ps_o", bufs=2, space="PSUM") as ps_o,
        ):
            # ---- constants ----
            w_sb = consts.tile([128, 3, 3 * C], BF16)  # wqkvT chunks
            nc.sync.dma_start(
                out=w_sb, in_=wqkvT[:].rearrange("(a p) c -> p a c", p=128)
            )
            wp_sb = consts.tile([128, 3, C], BF16)
            nc.sync.dma_start(
                out=wp_sb, in_=wprojT[:].rearrange("(a p) c -> p a c", p=128)
            )
            bqk_sb = consts.tile([128, 6], F32)  # per-partition bias cols
            nc.sync.dma_start(
                out=bqk_sb,
                in_=bass.AP(tensor=bqkv, offset=0, ap=[[1, 128], [128, 6]]),
            )
            bv_bc = consts.tile([128, C], F32)  # V bias broadcast over partitions
            nc.sync.dma_start(
                out=bv_bc,
                in_=bass.AP(tensor=bqkv, offset=2 * C, ap=[[0, 128], [1, C]]),
            )
            bp_bc = consts.tile([128, C], F32)
            nc.sync.dma_start(
                out=bp_bc,
                in_=bass.AP(tensor=bproj, offset=0, ap=[[0, 128], [1, C]]),
            )
            # prime DVE's observed DMA ticks so per-batch evac ops carry only
            # the PE wait (the DVE TT ISA struct has a single wait slot).
            for i, cst in enumerate((bqk_sb, bv_bc, bp_bc)):
                scratch = consts.tile([128, 1], F32, tag=f"scr{i}")
                nc.vector.tensor_copy(scratch, cst[:, 0:1])

            ident = consts.tile([128, 128], BF16)
            make_identity(nc, ident)
            # tri_neg[k, q] = NEG where k > q else 0  (future positions)
            tri_neg = consts.tile([128, 128], BF16)
            nc.gpsimd.memset(tri_neg, 0.0)
            nc.gpsimd.affine_select(
                out=tri_neg,
                in_=tri_neg,
                compare_op=mybir.AluOpType.is_ge,
                fill=NEG,
                base=0,
                pattern=[[1, 128]],
                channel_multiplier=-1,
            )

            for b in range(BPC):
                # ---- load x^T slice for this batch ----
                xb = xbp.tile([128, 3, T], BF16)
                nc.sync.dma_start(
                    out=xb,
                    in_=bass.AP(
                        tensor=xT,
                        offset=b * T,
                        ap=[[NTOK, 128], [128 * NTOK, 3], [1, T]],
                    ),
                )

                # ---- QKV projection ----
                # Q^T,K^T feature-major: [128, 6 chunks, 256]
                qk_sb = qkp.tile([128, 6, T], BF16)
                for f in range(6):
                    ps_f = ps_gen.tile([128, T], F32, tag="gen")
                    for ci in range(3):
                        nc.tensor.matmul(
                            ps_f,
                            lhsT=w_sb[:, ci, f * 128 : (f + 1) * 128],
                            rhs=xb[:, ci, :],
                            start=(ci == 0),
                            stop=(ci == 2),
                        )
                    nc.vector.tensor_tensor(
                        out=qk_sb[:, f, :],
                        in0=ps_f,
                        in1=bqk_sb[:, f : f + 1].to_broadcast((128, T)),
                        op=mybir.AluOpType.add,
                    )
                # V token-major with ones column: [128, 2, 6, 65]
                v_sb = vp.tile([128, 2, H, DH + 1], BF16)
                nc.vector.memset(v_sb[:, :, :, DH : DH + 1], 1.0)
                for tc_i in range(2):
                    ps_v = ps_gen.tile([128, C], F32, tag="gen")
                    for ci in range(3):
                        nc.tensor.matmul(
                            ps_v,
                            lhsT=xb[:, ci, tc_i * 128 : (tc_i + 1) * 128],
                            rhs=w_sb[:, ci, 2 * C : 3 * C],
                            start=(ci == 0),
                            stop=(ci == 2),
                        )
                    nc.vector.scalar_tensor_tensor(
                        out=v_sb[:, tc_i, :, 0:DH],
                        in0=ps_v.rearrange("p (h d) -> p h d", h=H),
                        scalar=0.0,
                        in1=bv_bc.rearrange("p (h d) -> p h d", h=H),
                        op0=mybir.AluOpType.add,
                        op1=mybir.AluOpType.add,
                    )

                # ---- S^T = K @ Q^T per head (packed 2 heads per psum pair) ----
                # pair psum [128, 768]: head even -> cols 0:384, odd -> 384:768
                # per head: kc0 q0 [0:128] (tri-masked), kc0 q1 [128:256],
                #           kc1 q1 [256:384] (tri-masked)
                p_sb = pp.tile([128, 3, 768], BF16)
                for j in range(3):
                    ps_pair = ps_s.tile([128, 768], F32)
                    for m in range(2):
                        h = 2 * j + m
                        po = m * 64  # partition offset within chunk
                        base = m * 384
                        kT = qk_sb[po : po + 64, 3 + j, :]
                        qT = qk_sb[po : po + 64, h // 2, :]
                        # kc0 x q0 (diagonal block, masked)
                        nc.tensor.matmul(
                            ps_pair[:, base : base + 128],
                            lhsT=kT[:, 0:128],
                            rhs=qT[:, 0:128],
                            start=True,
                            stop=False,
                        )
                        nc.tensor.matmul(
                            ps_pair[:, base : base + 128],
                            lhsT=ident,
                            rhs=tri_neg,
                            start=False,
                            stop=True,
                        )
                        # kc0 x q1 (full block)
                        nc.tensor.matmul(
                            ps_pair[:, base + 128 : base + 256],
                            lhsT=kT[:, 0:128],
                            rhs=qT[:, 128:256],
                            start=True,
                            stop=True,
                        )
                        # kc1 x q1 (diagonal block, masked)
                        nc.tensor.matmul(
                            ps_pair[:, base + 256 : base + 384],
                            lhsT=kT[:, 128:256],
                            rhs=qT[:, 128:256],
                            start=True,
                            stop=False,
                        )
                        nc.tensor.matmul(
                            ps_pair[:, base + 256 : base + 384],
                            lhsT=ident,
                            rhs=tri_neg,
                            start=False,
                            stop=True,
                        )
                    # exp over the whole pair tile
                    nc.scalar.activation(
                        out=p_sb[:, j, :], in_=ps_pair, func=mybir.ActivationFunctionType.Exp,
                        scale=SCALE,
                    )

                # ---- PV: O[q, h*65+d] with rowsum col; normalize; transpose ----
                oT_sb = otp.tile([128, 3, T], BF16)
                for qc in range(2):
                    ps_pv = ps_o.tile([128, H * (DH + 1)], F32)
                    for h in range(H):
                        j, m = h // 2, h % 2
                        base = m * 384
                        pcol = h * (DH + 1)
                        # kc0 contribution (always)
                        nc.tensor.matmul(
                            ps_pv[:, pcol : pcol + DH + 1],
                            lhsT=p_sb[:, j, base + qc * 128 : base + qc * 128 + 128],
                            rhs=v_sb[:, 0, h, :],
                            start=True,
                            stop=(qc == 0),
                        )
                        if qc == 1:
                            nc.tensor.matmul(
                                ps_pv[:, pcol : pcol + DH + 1],
                                lhsT=p_sb[:, j, base + 256 : base + 384],
                                rhs=v_sb[:, 1, h, :],
                                start=False,
                                stop=True,
                            )
                    # reciprocal of rowsums (strided cols 64::65)
                    rc = op_pool.tile([128, H], F32, tag="rc")
                    nc.vector.reciprocal(
                        rc,
                        bass.AP(
                            tensor=ps_pv.tensor,
                            offset=ps_pv.offset + DH,
                            ap=[list(ps_pv.ap[0]), [DH + 1, H]],
                        ),
                    )
                    # normalized O (token-major) [128, 384] bf16
                    o_sb = op_pool.tile([128, H, DH], BF16, tag="o")
                    nc.vector.tensor_tensor(
                        out=o_sb,
                        in0=ps_pv.rearrange("p (h d) -> p h d", h=H)[:, :, 0:DH],
                        in1=rc.to_broadcast((128, H, DH)),
                        op=mybir.AluOpType.mult,
                    )
                    # transpose 3 chunks of [128,128] -> O^T feature-major
                    for ci in range(3):
                        ps_tr = ps_gen.tile([128, 128], BF16, tag="gen")
                        nc.tensor.transpose(
                            ps_tr,
                            o_sb.rearrange("p h d -> p (h d)")[
                                :, ci * 128 : (ci + 1) * 128
                            ],
                            ident,
                        )
                        nc.scalar.activation(
                            out=oT_sb[:, ci, qc * 128 : (qc + 1) * 128],
                            in_=ps_tr,
                            func=mybir.ActivationFunctionType.Copy,
                        )

                # ---- projection + bias + store ----
                for qc in range(2):
                    ps_y = ps_gen.tile([128, C], F32, tag="gen")
                    for ci in range(3):
                        nc.tensor.matmul(
                            ps_y,
                            lhsT=oT_sb[:, ci, qc * 128 : (qc + 1) * 128],
                            rhs=wp_sb[:, ci, :],
                            start=(ci == 0),
                            stop=(ci == 2),
                        )
                    y_sb = yp.tile([128, C], F32)
                    nc.vector.scalar_tensor_tensor(
                        out=y_sb,
                        in0=ps_y,
                        scalar=0.0,
                        in1=bp_bc,
                        op0=mybir.AluOpType.add,
                        op1=mybir.AluOpType.add,
                    )
                    nc.sync.dma_start(
                        out=out[b * T + qc * 128 : b * T + qc * 128 + 128, :],
                        in_=y_sb,
                    )
    nc.compile()
    return nc


def kernel(x, W_qkv, b_qkv, W_proj, b_proj):
    x = np.asarray(x, dtype=np.float32)
    W_qkv = np.asarray(W_qkv, dtype=np.float32)
    b_qkv = np.asarray(b_qkv, dtype=np.float32)
    W_proj = np.asarray(W_proj, dtype=np.float32)
    b_proj = np.asarray(b_proj, dtype=np.float32)

    if "nc" not in _CACHE:
        _CACHE["nc"] = build_kernel()
    nc = _CACHE["nc"]

    wqkvT = np.ascontiguousarray(W_qkv.T).astype(ml_dtypes.bfloat16)
    wprojT = np.ascontiguousarray(W_proj.T).astype(ml_dtypes.bfloat16)
    in_maps = []
    for i in range(N_CORES):
        xs = x[i * BPC : (i + 1) * BPC].reshape(NTOK, C)
        xTl = np.ascontiguousarray(xs.T).astype(ml_dtypes.bfloat16)
        in_maps.append(
            {
                "xT": xTl,
                "wqkvT": wqkvT,
                "wprojT": wprojT,
                "bqkv": b_qkv,
                "bproj": b_proj,
            }
        )
    res = run_bass_kernel_spmd(nc, in_maps, core_ids=list(range(N_CORES)))
    outs = [res.results[i]["out"].reshape(BPC, T, C) for i in range(N_CORES)]
    return np.concatenate(outs, axis=0).astype(np.float32)


# revision 15
# speedup vs baseline: 1.0942x; 1.0942x over previous
"""Trainium2 Bass kernel for causal multi-head attention block.

B=128, T=256, C=384, H=6, Dh=64. Data-parallel over batch: 16 batches per
core on 8 NeuronCores. Weights replicated; no collectives.

Per-core dataflow (all feature-major until the PV output):
  XT [384, 4096] bf16 (host pre-transposed)
  QKV^T: per batch, Q^T/K^T feature-major [768, 256], V token-major [256, 384+1]
  S^T[k,q] = K Q^T per head (causal-skipped), mask added via identity-matmul
  P = exp(S^T * 1/8) on ACT (no max subtraction: |S*scale| <= ~3)
  O[q, 65] = P^T.T @ V_aug  (P-stationary) -> rowsum in col 64
  normalize per-partition (q), PE-transpose O -> O^T, proj, +bias, DMA out.
"""

import sys

sys.path.insert(0, "/opt/trn_rl_repo")

import numpy as np
import ml_dtypes

import concourse.bass as bass
import concourse.mybir as mybir
import concourse.tile as tile
from concourse import bacc
from concourse.bass_utils import run_bass_kernel_spmd
from concourse.masks import make_identity

BF16 = mybir.dt.bfloat16
F32 = mybir.dt.float32

N_CORES = 8
B_FULL, T, C = 128, 256, 384
H, DH = 6, 64
BPC = B_FULL // N_CORES  # 16 batches per core
NTOK = BPC * T  # 4096 tokens per core
SCALE = 1.0 / 8.0  # 1/sqrt(64)
NEG = -30000.0  # mask add; exp(NEG*SCALE) == 0 in f32/bf16

_CACHE = {}


def build_kernel():
    nc = bacc.Bacc()
    xT = nc.declare_dram_parameter("xT", [C, NTOK], BF16, isOutput=False)
    wqkvT = nc.declare_dram_parameter("wqkvT", [C, 3 * C], BF16, isOutput=False)
    wprojT = nc.declare_dram_parameter("wprojT", [C, C], BF16, isOutput=False)
    bqkv = nc.declare_dram_parameter("bqkv", [3 * C], F32, isOutput=False)
    bproj = nc.declare_dram_parameter("bproj", [C], F32, isOutput=False)
    out = nc.declare_dram_parameter("out", [NTOK, C], F32, isOutput=True)

    with tile.TileContext(nc) as tc:
        with (
            tc.tile_pool(name="consts", bufs=1) as consts,
            tc.tile_pool(name="xbp", bufs=3) as xbp,
            tc.tile_pool(name="qkp", bufs=2) as qkp,
            tc.tile_pool(name="vp", bufs=2) as vp,
            tc.tile_pool(name="pp", bufs=2) as pp,
            tc.tile_pool(name="op", bufs=3) as op_pool,
            tc.tile_pool(name="otp", bufs=2) as otp,
            tc.tile_pool(name="yp", bufs=3) as yp,
            tc.tile_pool(name="ps_gen", bufs=2, space="PSUM") as ps_gen,
            tc.tile_pool(name="ps_s", bufs=2, space="PSUM") as ps_s,
            tc.tile_pool(name="ps_o", bufs=2, space="PSUM") as ps_o,
        ):
            # ---- constants ----
            w_sb = consts.tile([128, 3, 3 * C], BF16)  # wqkvT chunks
            nc.sync.dma_start(
                out=w_sb, in_=wqkvT[:].rearrange("(a p) c -> p a c", p=128)
            )
            wp_sb = consts.tile([128, 3, C], BF16)
            nc.sync.dma_start(
                out=wp_sb, in_=wprojT[:].rearrange("(a p) c -> p a c", p=128)
            )
            bqk_sb = consts.tile([128, 6], F32)  # per-partition bias cols
            nc.sync.dma_start(
                out=bqk_sb,
                in_=bass.AP(tensor=bqkv, offset=0, ap=[[1, 128], [128, 6]]),
            )
            bv_bc = consts.tile([128, C], F32)  # V bias broadcast over partitions
            nc.sync.dma_start(
                out=bv_bc,
                in_=bass.AP(tensor=bqkv, offset=2 * C, ap=[[0, 128], [1, C]]),
            )
            bp_bc = consts.tile([128, C], F32)
            nc.sync.dma_start(
                out=bp_bc,
                in_=bass.AP(tensor=bproj, offset=0, ap=[[0, 128], [1, C]]),
            )
            # prime DVE's observed DMA ticks so per-batch evac ops carry only
            # the PE wait (the DVE TT ISA struct has a single wait slot).
            for i, cst in enumerate((bqk_sb, bv_bc, bp_bc)):
                scratch = consts.tile([128, 1], F32, tag=f"scr{i}")
                nc.vector.tensor_copy(scratch, cst[:, 0:1])

            ident = consts.tile([128, 128], BF16)
            make_identity(nc, ident)
            # tri01[k, q] = 1 where k <= q else 0 (multiplicative causal mask)
            tri01 = consts.tile([128, 128], BF16)
            nc.gpsimd.memset(tri01, 1.0)
            nc.gpsimd.affine_select(
                out=tri01,
                in_=tri01,
                compare_op=mybir.AluOpType.is_ge,
                fill=0.0,
                base=0,
                pattern=[[1, 128]],
                channel_multiplier=-1,
            )

            for b in range(BPC):
                # ---- load x^T slice for this batch ----
                xb = xbp.tile([128, 3, T], BF16)
                nc.sync.dma_start(
                    out=xb,
                    in_=bass.AP(
                        tensor=xT,
                        offset=b * T,
                        ap=[[NTOK, 128], [128 * NTOK, 3], [1, T]],
                    ),
                )

                # ---- QKV projection ----
                # Q^T,K^T feature-major: [128, 6 chunks, 256]
                qk_sb = qkp.tile([128, 6, T], BF16)
                for f in range(6):
                    ps_f = ps_gen.tile([128, T], F32, tag="gen")
                    for ci in range(3):
                        nc.tensor.matmul(
                            ps_f,
                            lhsT=w_sb[:, ci, f * 128 : (f + 1) * 128],
                            rhs=xb[:, ci, :],
                            start=(ci == 0),
                            stop=(ci == 2),
                        )
                    nc.vector.tensor_tensor(
                        out=qk_sb[:, f, :],
                        in0=ps_f,
                        in1=bqk_sb[:, f : f + 1].to_broadcast((128, T)),
                        op=mybir.AluOpType.add,
                    )
                # V token-major with ones column: [128, 2, 6, 65]
                v_sb = vp.tile([128, 2, H, DH + 1], BF16)
                nc.vector.memset(v_sb[:, :, :, DH : DH + 1], 1.0)
                for tc_i in range(2):
                    ps_v = ps_gen.tile([128, C], F32, tag="gen")
                    for ci in range(3):
                        nc.tensor.matmul(
                            ps_v,
                            lhsT=xb[:, ci, tc_i * 128 : (tc_i + 1) * 128],
                            rhs=w_sb[:, ci, 2 * C : 3 * C],
                            start=(ci == 0),
                            stop=(ci == 2),
                        )
                    nc.vector.scalar_tensor_tensor(
                        out=v_sb[:, tc_i, :, 0:DH],
                        in0=ps_v.rearrange("p (h d) -> p h d", h=H),
                        scalar=0.0,
                        in1=bv_bc.rearrange("p (h d) -> p h d", h=H),
                        op0=mybir.AluOpType.add,
                        op1=mybir.AluOpType.add,
                    )

                # ---- S^T = K @ Q^T per head ----
                # per-head psum [128, 384]: kc0 x q[0:256] at 0:256,
                # kc1 x q1 at 256:384. Heads 2j/2j+1 sit on array row
                # groups 0:64 / 64:128 -> their MMs and LDWs overlap.
                p_sb = pp.tile([128, H, 384], BF16)
                for j in range(3):
                    ps_h = [ps_s.tile([128, 384], F32, tag=f"s{m}", name=f"ps_h{m}") for m in range(2)]
                    for kc in range(2):
                        for m in range(2):
                            h = 2 * j + m
                            po = m * 64
                            kT = qk_sb[po : po + 64, 3 + j, :]
                            qT = qk_sb[po : po + 64, h // 2, :]
                            if kc == 0:
                                nc.tensor.matmul(
                                    ps_h[m][:, 0:256],
                                    lhsT=kT[:, 0:128],
                                    rhs=qT,
                                    start=True,
                                    stop=True,
                                )
                            else:
                                nc.tensor.matmul(
                                    ps_h[m][:, 256:384],
                                    lhsT=kT[:, 128:256],
                                    rhs=qT[:, 128:256],
                                    start=True,
                                    stop=True,
                                )
                    for m in range(2):
                        h = 2 * j + m
                        nc.scalar.activation(
                            out=p_sb[:, h, :],
                            in_=ps_h[m],
                            func=mybir.ActivationFunctionType.Exp,
                            scale=SCALE,
                        )
                        # causal mask: zero future positions in the two
                        # diagonal blocks (cols 0:128 and 256:384)
                        dv = bass.AP(
                            tensor=p_sb.tensor,
                            offset=p_sb.offset + h * 384,
                            ap=[list(p_sb.ap[0]), [256, 2], [1, 128]],
                        )
                        nc.vector.tensor_tensor(
                            out=dv,
                            in0=dv,
                            in1=bass.AP(
                                tensor=tri01.tensor,
                                offset=tri01.offset,
                                ap=[list(tri01.ap[0]), [0, 2], [1, 128]],
                            ),
                            op=mybir.AluOpType.mult,
                        )

                # ---- PV: O[q, h*65+d] with rowsum col; normalize; transpose ----
                oT_sb = otp.tile([128, 3, T], BF16)
                for qc in range(2):
                    ps_pv = ps_o.tile([128, H * (DH + 1)], F32)
                    for h in range(H):
                        pcol = h * (DH + 1)
                        # kc0 contribution (always)
                        nc.tensor.matmul(
                            ps_pv[:, pcol : pcol + DH + 1],
                            lhsT=p_sb[:, h, qc * 128 : qc * 128 + 128],
                            rhs=v_sb[:, 0, h, :],
                            start=True,
                            stop=(qc == 0),
                        )
                        if qc == 1:
                            nc.tensor.matmul(
                                ps_pv[:, pcol : pcol + DH + 1],
                                lhsT=p_sb[:, h, 256:384],
                                rhs=v_sb[:, 1, h, :],
                                start=False,
                                stop=True,
                            )
                    # reciprocal of rowsums (strided cols 64::65)
                    rc = op_pool.tile([128, H], F32, tag="rc")
                    nc.vector.reciprocal(
                        rc,
                        bass.AP(
                            tensor=ps_pv.tensor,
                            offset=ps_pv.offset + DH,
                            ap=[list(ps_pv.ap[0]), [DH + 1, H]],
                        ),
                    )
                    # normalized O (token-major) [128, 384] bf16
                    o_sb = op_pool.tile([128, H, DH], BF16, tag="o")
                    nc.vector.tensor_tensor(
                        out=o_sb,
                        in0=ps_pv.rearrange("p (h d) -> p h d", h=H)[:, :, 0:DH],
                        in1=rc.to_broadcast((128, H, DH)),
                        op=mybir.AluOpType.mult,
                    )
                    # transpose 3 chunks of [128,128] -> O^T feature-major
                    for ci in range(3):
                        ps_tr = ps_gen.tile([128, 128], BF16, tag="gen")
                        nc.tensor.transpose(
                            ps_tr,
                            o_sb.rearrange("p h d -> p (h d)")[
                                :, ci * 128 : (ci + 1) * 128
                            ],
                            ident,
                        )
                        nc.scalar.activation(
                            out=oT_sb[:, ci, qc * 128 : (qc + 1) * 128],
                            in_=ps_tr,
                            func=mybir.ActivationFunctionType.Copy,
                        )

                # ---- projection + bias + store ----
                for qc in range(2):
                    ps_y = ps_gen.tile([128, C], F32, tag="gen")
                    for ci in range(3):
                        nc.tensor.matmul(
                            ps_y,
                            lhsT=oT_sb[:, ci, qc * 128 : (qc + 1) * 128],
                            rhs=wp_sb[:, ci, :],
                            start=(ci == 0),
                            stop=(ci == 2),
                        )
                    y_sb = yp.tile([128, C], F32)
                    nc.vector.scalar_tensor_tensor(
                        out=y_sb,
                        in0=ps_y,
                        scalar=0.0,
                        in1=bp_bc,
                        op0=mybir.AluOpType.add,
                        op1=mybir.AluOpType.add,
                    )
                    nc.sync.dma_start(
                        out=out[b * T + qc * 128 : b * T + qc * 128 + 128, :],
                        in_=y_sb,
                    )
    nc.compile()
    return nc


def kernel(x, W_qkv, b_qkv, W_proj, b_proj):
    x = np.asarray(x, dtype=np.float32)
    W_qkv = np.asarray(W_qkv, dtype=np.float32)
    b_qkv = np.asarray(b_qkv, dtype=np.float32)
    W_proj = np.asarray(W_proj, dtype=np.float32)
    b_proj = np.asarray(b_proj, dtype=np.float32)

    if "nc" not in _CACHE:
        _CACHE["nc"] = build_kernel()
    nc = _CACHE["nc"]

    wqkvT = np.ascontiguousarray(W_qkv.T).astype(ml_dtypes.bfloat16)
    wprojT = np.ascontiguousarray(W_proj.T).astype(ml_dtypes.bfloat16)
    in_maps = []
    for i in range(N_CORES):
        xs = x[i * BPC : (i + 1) * BPC].reshape(NTOK, C)
        xTl = np.ascontiguousarray(xs.T).astype(ml_dtypes.bfloat16)
        in_maps.append(
            {
                "xT": xTl,
                "wqkvT": wqkvT,
                "wprojT": wprojT,
                "bqkv": b_qkv,
                "bproj": b_proj,
            }
        )
    res = run_bass_kernel_spmd(nc, in_maps, core_ids=list(range(N_CORES)))
    outs = [res.results[i]["out"].reshape(BPC, T, C) for i in range(N_CORES)]
    return np.concatenate(outs, axis=0).astype(np.float32)


# revision 19
# speedup vs baseline: 1.2105x; 1.1063x over previous
"""Trainium2 Bass kernel for causal multi-head attention block.

B=128, T=256, C=384, H=6, Dh=64. Data-parallel over batch: 16 batches per
core on 8 NeuronCores. Weights replicated; no collectives.

Per-core dataflow (all feature-major until the PV output):
  XT [384, 4096] bf16 (host pre-transposed)
  QKV^T: per batch, Q^T/K^T feature-major [768, 256], V token-major [256, 384+1]
  S^T[k,q] = K Q^T per head (causal-skipped), mask added via identity-matmul
  P = exp(S^T * 1/8) on ACT (no max subtraction: |S*scale| <= ~3)
  O[q, 65] = P^T.T @ V_aug  (P-stationary) -> rowsum in col 64
  normalize per-partition (q), PE-transpose O -> O^T, proj, +bias, DMA out.
"""

import sys

sys.path.insert(0, "/opt/trn_rl_repo")

import numpy as np
import ml_dtypes

import concourse.bass as bass
import concourse.mybir as mybir
import concourse.tile as tile
from concourse import bacc
from concourse.bass_utils import run_bass_kernel_spmd
from concourse.masks import make_identity

BF16 = mybir.dt.bfloat16
F32 = mybir.dt.float32

N_CORES = 8
B_FULL, T, C = 128, 256, 384
H, DH = 6, 64
BPC = B_FULL // N_CORES  # 16 batches per core
NTOK = BPC * T  # 4096 tokens per core
SCALE = 1.0 / 8.0  # 1/sqrt(64)
NEG = -30000.0  # mask add; exp(NEG*SCALE) == 0 in f32/bf16

_CACHE = {}


def build_kernel():
    nc = bacc.Bacc()
    xT = nc.declare_dram_parameter("xT", [C, NTOK], BF16, isOutput=False)
    wqkvT = nc.declare_dram_parameter("wqkvT", [C, 3 * C], BF16, isOutput=False)
    wprojT = nc.declare_dram_parameter("wprojT", [C, C], BF16, isOutput=False)
    bqkv = nc.declare_dram_parameter("bqkv", [3 * C], F32, isOutput=False)
    bproj = nc.declare_dram_parameter("bproj", [C], F32, isOutput=False)
    out = nc.declare_dram_parameter("out", [NTOK, C], F32, isOutput=True)

    with tile.TileContext(nc) as tc:
        with (
            tc.tile_pool(name="consts", bufs=1) as consts,
            tc.tile_pool(name="xbp", bufs=3) as xbp,
            tc.tile_pool(name="qkp", bufs=2) as qkp,
            tc.tile_pool(name="vp", bufs=2) as vp,
            tc.tile_pool(name="pp", bufs=2) as pp,
            tc.tile_pool(name="op", bufs=3) as op_pool,
            tc.tile_pool(name="otp", bufs=2) as otp,
            tc.tile_pool(name="yp", bufs=3) as yp,
            tc.tile_pool(name="ps_gen", bufs=3, space="PSUM") as ps_gen,
            tc.tile_pool(name="ps_s", bufs=2, space="PSUM") as ps_s,
            tc.tile_pool(name="ps_o", bufs=2, space="PSUM") as ps_o,
        ):
            # ---- constants ----
            w_sb = consts.tile([128, 3, 3 * C], BF16)  # wqkvT chunks
            nc.sync.dma_start(
                out=w_sb, in_=wqkvT[:].rearrange("(a p) c -> p a c", p=128)
            )
            wp_sb = consts.tile([128, 3, C], BF16)
            nc.sync.dma_start(
                out=wp_sb, in_=wprojT[:].rearrange("(a p) c -> p a c", p=128)
            )
            bqk_sb = consts.tile([128, 6], F32)  # per-partition bias cols
            nc.sync.dma_start(
                out=bqk_sb,
                in_=bass.AP(tensor=bqkv, offset=0, ap=[[1, 128], [128, 6]]),
            )
            bv_bc = consts.tile([128, C], F32)  # V bias broadcast over partitions
            nc.sync.dma_start(
                out=bv_bc,
                in_=bass.AP(tensor=bqkv, offset=2 * C, ap=[[0, 128], [1, C]]),
            )
            bp_bc = consts.tile([128, C], F32)
            nc.sync.dma_start(
                out=bp_bc,
                in_=bass.AP(tensor=bproj, offset=0, ap=[[0, 128], [1, C]]),
            )
            # prime DVE's observed DMA ticks so per-batch evac ops carry only
            # the PE wait (the DVE TT ISA struct has a single wait slot).
            for i, cst in enumerate((bqk_sb, bv_bc, bp_bc)):
                scratch = consts.tile([128, 1], F32, tag=f"scr{i}")
                nc.vector.tensor_copy(scratch, cst[:, 0:1])

            ident = consts.tile([128, 128], BF16)
            make_identity(nc, ident)
            # tri01[k, q] = 1 where k <= q else 0 (multiplicative causal mask)
            tri01 = consts.tile([128, 128], BF16)
            nc.gpsimd.memset(tri01, 1.0)
            nc.gpsimd.affine_select(
                out=tri01,
                in_=tri01,
                compare_op=mybir.AluOpType.is_ge,
                fill=0.0,
                base=0,
                pattern=[[1, 128]],
                channel_multiplier=-1,
            )

            for b in range(BPC):
                # ---- load x^T slice for this batch ----
                xb = xbp.tile([128, 3, T], BF16)
                nc.sync.dma_start(
                    out=xb,
                    in_=bass.AP(
                        tensor=xT,
                        offset=b * T,
                        ap=[[NTOK, 128], [128 * NTOK, 3], [1, T]],
                    ),
                )

                # ---- QKV projection ----
                # Q^T,K^T feature-major: [128, 6 chunks, 256]
                qk_sb = qkp.tile([128, 6, T], BF16)
                for f in range(0, 6, 2):
                    ps_f = ps_gen.tile([128, 2, T], F32, tag="gen")
                    for g in range(2):
                        for ci in range(3):
                            nc.tensor.matmul(
                                ps_f[:, g, :],
                                lhsT=w_sb[:, ci, (f + g) * 128 : (f + g + 1) * 128],
                                rhs=xb[:, ci, :],
                                start=(ci == 0),
                                stop=(ci == 2),
                            )
                    nc.vector.tensor_tensor(
                        out=qk_sb[:, f : f + 2, :],
                        in0=ps_f,
                        in1=bqk_sb[:, f : f + 2].to_broadcast((128, 2, T)),
                        op=mybir.AluOpType.add,
                    )
                # V token-major with ones column: [128, 2, 6, 65]
                v_sb = vp.tile([128, 2, H, DH + 1], BF16)
                nc.vector.memset(v_sb[:, :, :, DH : DH + 1], 1.0)
                for tc_i in range(2):
                    ps_v = ps_gen.tile([128, C], F32, tag="gen")
                    for ci in range(3):
                        nc.tensor.matmul(
                            ps_v,
                            lhsT=xb[:, ci, tc_i * 128 : (tc_i + 1) * 128],
                            rhs=w_sb[:, ci, 2 * C : 3 * C],
                            start=(ci == 0),
                            stop=(ci == 2),
                        )
                    nc.vector.scalar_tensor_tensor(
                        out=v_sb[:, tc_i, :, 0:DH],
                        in0=ps_v.rearrange("p (h d) -> p h d", h=H),
                        scalar=0.0,
                        in1=bv_bc.rearrange("p (h d) -> p h d", h=H),
                        op0=mybir.AluOpType.add,
                        op1=mybir.AluOpType.add,
                    )

                # ---- S^T = K @ Q^T per head ----
                # per-head psum [128, 384]: kc0 x q[0:256] at 0:256,
                # kc1 x q1 at 256:384. Heads 2j/2j+1 sit on array row
                # groups 0:64 / 64:128 -> their MMs and LDWs overlap.
                p_sb = pp.tile([128, H, 384], BF16)
                for j in range(3):
                    ps_h = [ps_s.tile([128, 384], F32, tag="s", name=f"ps_h{m}") for m in range(2)]
                    for kc in range(2):
                        for m in range(2):
                            h = 2 * j + m
                            po = m * 64
                            kT = qk_sb[po : po + 64, 3 + j, :]
                            qT = qk_sb[po : po + 64, h // 2, :]
                            if kc == 0:
                                nc.tensor.matmul(
                                    ps_h[m][:, 0:256],
                                    lhsT=kT[:, 0:128],
                                    rhs=qT,
                                    start=True,
                                    stop=True,
                                )
                            else:
                                nc.tensor.matmul(
                                    ps_h[m][:, 256:384],
                                    lhsT=kT[:, 128:256],
                                    rhs=qT[:, 128:256],
                                    start=True,
                                    stop=True,
                                )
                    for m in range(2):
                        h = 2 * j + m
                        nc.scalar.activation(
                            out=p_sb[:, h, :],
                            in_=ps_h[m],
                            func=mybir.ActivationFunctionType.Exp,
                            scale=SCALE,
                        )
                        # causal mask: zero future positions in the two
                        # diagonal blocks (cols 0:128 and 256:384)
                        dv = bass.AP(
                            tensor=p_sb.tensor,
                            offset=p_sb.offset + h * 384,
                            ap=[list(p_sb.ap[0]), [256, 2], [1, 128]],
                        )
                        nc.vector.tensor_tensor(
                            out=dv,
                            in0=dv,
                            in1=bass.AP(
                                tensor=tri01.tensor,
                                offset=tri01.offset,
                                ap=[list(tri01.ap[0]), [0, 2], [1, 128]],
                            ),
                            op=mybir.AluOpType.mult,
                        )

                # ---- PV: O[q, h*65+d] with rowsum col; normalize; transpose ----
                oT_sb = otp.tile([128, 3, T], BF16)
                for qc in range(2):
                    ps_pv = ps_o.tile([128, H * (DH + 1)], F32)
                    for h in range(H):
                        pcol = h * (DH + 1)
                        # kc0 contribution (always)
                        nc.tensor.matmul(
                            ps_pv[:, pcol : pcol + DH + 1],
                            lhsT=p_sb[:, h, qc * 128 : qc * 128 + 128],
                            rhs=v_sb[:, 0, h, :],
                            start=True,
                            stop=(qc == 0),
                        )
                        if qc == 1:
                            nc.tensor.matmul(
                                ps_pv[:, pcol : pcol + DH + 1],
                                lhsT=p_sb[:, h, 256:384],
                                rhs=v_sb[:, 1, h, :],
                                start=False,
                                stop=True,
                            )
                    # reciprocal of rowsums (strided cols 64::65)
                    rc = op_pool.tile([128, H], F32, tag="rc")
                    nc.vector.reciprocal(
                        rc,
                        bass.AP(
                            tensor=ps_pv.tensor,
                            offset=ps_pv.offset + DH,
                            ap=[list(ps_pv.ap[0]), [DH + 1, H]],
                        ),
                    )
                    # normalized O (token-major) [128, 384] bf16
                    o_sb = op_pool.tile([128, H, DH], BF16, tag="o")
                    nc.vector.tensor_tensor(
                        out=o_sb,
                        in0=ps_pv.rearrange("p (h d) -> p h d", h=H)[:, :, 0:DH],
                        in1=rc.to_broadcast((128, H, DH)),
                        op=mybir.AluOpType.mult,
                    )
                    # transpose 3 chunks of [128,128] -> O^T feature-major
                    ps_tr = ps_o.tile([128, 3, 128], BF16, tag="tr", bufs=1)
                    for ci in range(3):
                        nc.tensor.transpose(
                            ps_tr[:, ci, :],
                            o_sb.rearrange("p h d -> p (h d)")[
                                :, ci * 128 : (ci + 1) * 128
                            ],
                            ident,
                        )
                    nc.scalar.activation(
                        out=bass.AP(
                            tensor=oT_sb.tensor,
                            offset=oT_sb.offset + qc * 128,
                            ap=[list(oT_sb.ap[0]), [T, 3], [1, 128]],
                        ),
                        in_=ps_tr,
                        func=mybir.ActivationFunctionType.Copy,
                    )

                # ---- projection + bias + store ----
                for qc in range(2):
                    ps_y = ps_gen.tile([128, C], F32, tag="gen")
                    for ci in range(3):
                        nc.tensor.matmul(
                            ps_y,
                            lhsT=oT_sb[:, ci, qc * 128 : (qc + 1) * 128],
                            rhs=wp_sb[:, ci, :],
                            start=(ci == 0),
                            stop=(ci == 2),
                        )
                    y_sb = yp.tile([128, C], F32)
                    nc.vector.scalar_tensor_tensor(
                        out=y_sb,
                        in0=ps_y,
                        scalar=0.0,
                        in1=bp_bc,
                        op0=mybir.AluOpType.add,
                        op1=mybir.AluOpType.add,
                    )
                    nc.sync.dma_start(
                        out=out[b * T + qc * 128 : b * T + qc * 128 + 128, :],
                        in_=y_sb,
                    )
    nc.compile()
    return nc


def kernel(x, W_qkv, b_qkv, W_proj, b_proj):
    x = np.asarray(x, dtype=np.float32)
    W_qkv = np.asarray(W_qkv, dtype=np.float32)
    b_qkv = np.asarray(b_qkv, dtype=np.float32)
    W_proj = np.asarray(W_proj, dtype=np.float32)
    b_proj = np.asarray(b_proj, dtype=np.float32)

    if "nc" not in _CACHE:
        _CACHE["nc"] = build_kernel()
    nc = _CACHE["nc"]

    wqkvT = np.ascontiguousarray(W_qkv.T).astype(ml_dtypes.bfloat16)
    wprojT = np.ascontiguousarray(W_proj.T).astype(ml_dtypes.bfloat16)
    in_maps = []
    for i in range(N_CORES):
        xs = x[i * BPC : (i + 1) * BPC].reshape(NTOK, C)
        xTl = np.ascontiguousarray(xs.T).astype(ml_dtypes.bfloat16)
        in_maps.append(
            {
                "xT": xTl,
                "wqkvT": wqkvT,
                "wprojT": wprojT,
                "bqkv": b_qkv,
                "bproj": b_proj,
            }
        )
    res = run_bass_kernel_spmd(nc, in_maps, core_ids=list(range(N_CORES)))
    outs = [res.results[i]["out"].reshape(BPC, T, C) for i in range(N_CORES)]
    return np.concatenate(outs, axis=0).astype(np.float32)


# revision 20
# speedup vs baseline: 1.2874x; 1.0635x over previous
"""Trainium2 Bass kernel for causal multi-head attention block.

B=128, T=256, C=384, H=6, Dh=64. Data-parallel over batch: 16 batches per
core on 8 NeuronCores. Weights replicated; no collectives.

Per-core dataflow (all feature-major until the PV output):
  XT [384, 4096] bf16 (host pre-transposed), processed 2 batches (512 tok)
  per iteration:
  QKV^T: Q^T/K^T feature-major [768, 512], V token-major [512, 384+1]
  S^T[k,q] = K Q^T per head/batch (causal-skipped)
  P = exp(S^T * 1/8) on ACT (no max subtraction; |S*scale| small),
  causal mask applied multiplicatively on GPSIMD
  O[q, 65] = P^T.T @ V_aug  (P-stationary) -> rowsum in col 64
  normalize per-partition (q), PE-transpose O -> O^T, proj, +bias, DMA out.
"""

import sys

sys.path.insert(0, "/opt/trn_rl_repo")

import numpy as np
import ml_dtypes

import concourse.bass as bass
import concourse.mybir as mybir
import concourse.tile as tile
from concourse import bacc
from concourse.bass_utils import run_bass_kernel_spmd
from concourse.masks import make_identity

BF16 = mybir.dt.bfloat16
F32 = mybir.dt.float32

N_CORES = 8
B_FULL, T, C = 128, 256, 384
H, DH = 6, 64
BPC = B_FULL // N_CORES  # 16 batches per core
NTOK = BPC * T  # 4096 tokens per core
SCALE = 1.0 / 8.0  # 1/sqrt(64)

_CACHE = {}


def build_kernel():
    nc = bacc.Bacc()
    xT = nc.declare_dram_parameter("xT", [C, NTOK], BF16, isOutput=False)
    wqkvT = nc.declare_dram_parameter("wqkvT", [C, 3 * C], BF16, isOutput=False)
    wprojT = nc.declare_dram_parameter("wprojT", [C, C], BF16, isOutput=False)
    bqkv = nc.declare_dram_parameter("bqkv", [3 * C], F32, isOutput=False)
    bproj = nc.declare_dram_parameter("bproj", [C], F32, isOutput=False)
    out = nc.declare_dram_parameter("out", [NTOK, C], F32, isOutput=True)

    T2 = 2 * T  # tokens per iteration (2 batches)

    with tile.TileContext(nc) as tc:
        with (
            tc.tile_pool(name="consts", bufs=1) as consts,
            tc.tile_pool(name="xbp", bufs=3) as xbp,
            tc.tile_pool(name="qkp", bufs=2) as qkp,
            tc.tile_pool(name="vp", bufs=2) as vp,
            tc.tile_pool(name="pp", bufs=3) as pp,
            tc.tile_pool(name="op", bufs=3) as op_pool,
            tc.tile_pool(name="otp", bufs=3) as otp,
            tc.tile_pool(name="yp", bufs=3) as yp,
            tc.tile_pool(name="ps_gen", bufs=3, space="PSUM") as ps_gen,
            tc.tile_pool(name="ps_s", bufs=2, space="PSUM") as ps_s,
            tc.tile_pool(name="ps_o", bufs=2, space="PSUM") as ps_o,
        ):
            # ---- constants ----
            w_sb = consts.tile([128, 3, 3 * C], BF16)  # wqkvT chunks
            nc.sync.dma_start(
                out=w_sb, in_=wqkvT[:].rearrange("(a p) c -> p a c", p=128)
            )
            wp_sb = consts.tile([128, 3, C], BF16)
            nc.sync.dma_start(
                out=wp_sb, in_=wprojT[:].rearrange("(a p) c -> p a c", p=128)
            )
            bqk_sb = consts.tile([128, 6], F32)  # per-partition bias cols
            nc.sync.dma_start(
                out=bqk_sb,
                in_=bass.AP(tensor=bqkv, offset=0, ap=[[1, 128], [128, 6]]),
            )
            bv_bc = consts.tile([128, C], F32)  # V bias broadcast over partitions
            nc.sync.dma_start(
                out=bv_bc,
                in_=bass.AP(tensor=bqkv, offset=2 * C, ap=[[0, 128], [1, C]]),
            )
            bp_bc = consts.tile([128, C], F32)
            nc.sync.dma_start(
                out=bp_bc,
                in_=bass.AP(tensor=bproj, offset=0, ap=[[0, 128], [1, C]]),
            )
            # prime DVE's observed DMA ticks so per-batch evac ops carry only
            # the PE wait (the DVE TT ISA struct has a single wait slot).
            for i, cst in enumerate((bqk_sb, bv_bc, bp_bc)):
                scratch = consts.tile([128, 1], F32, tag=f"scr{i}")
                nc.vector.tensor_copy(scratch, cst[:, 0:1])

            ident = consts.tile([128, 128], BF16)
            make_identity(nc, ident)
            # tri01[k, q] = 1 where k <= q else 0 (multiplicative causal mask)
            tri01 = consts.tile([128, 128], BF16)
            nc.gpsimd.memset(tri01, 1.0)
            nc.gpsimd.affine_select(
                out=tri01,
                in_=tri01,
                compare_op=mybir.AluOpType.is_ge,
                fill=0.0,
                base=0,
                pattern=[[1, 128]],
                channel_multiplier=-1,
            )

            for bb in range(BPC // 2):
                # ---- load x^T slice for this batch pair ----
                xb = xbp.tile([128, 3, T2], BF16)
                nc.sync.dma_start(
                    out=xb,
                    in_=bass.AP(
                        tensor=xT,
                        offset=bb * T2,
                        ap=[[NTOK, 128], [128 * NTOK, 3], [1, T2]],
                    ),
                )

                # ---- QKV projection (2 batches at once) ----
                qk_sb = qkp.tile([128, 6, T2], BF16)
                for f in range(6):
                    ps_f = ps_gen.tile([128, T2], F32, tag="gen")
                    for ci in range(3):
                        nc.tensor.matmul(
                            ps_f,
                            lhsT=w_sb[:, ci, f * 128 : (f + 1) * 128],
                            rhs=xb[:, ci, :],
                            start=(ci == 0),
                            stop=(ci == 2),
                        )
                    nc.vector.tensor_tensor(
                        out=qk_sb[:, f, :],
                        in0=ps_f,
                        in1=bqk_sb[:, f : f + 1].to_broadcast((128, T2)),
                        op=mybir.AluOpType.add,
                    )
                # V token-major with ones column: [128, 4 tok-chunks, 6, 65]
                v_sb = vp.tile([128, 4, H, DH + 1], BF16)
                nc.vector.memset(v_sb[:, :, :, DH : DH + 1], 1.0)
                for tc_i in range(4):
                    ps_v = ps_gen.tile([128, C], F32, tag="gen")
                    for ci in range(3):
                        nc.tensor.matmul(
                            ps_v,
                            lhsT=xb[:, ci, tc_i * 128 : (tc_i + 1) * 128],
                            rhs=w_sb[:, ci, 2 * C : 3 * C],
                            start=(ci == 0),
                            stop=(ci == 2),
                        )
                    nc.vector.scalar_tensor_tensor(
                        out=v_sb[:, tc_i, :, 0:DH],
                        in0=ps_v.rearrange("p (h d) -> p h d", h=H),
                        scalar=0.0,
                        in1=bv_bc.rearrange("p (h d) -> p h d", h=H),
                        op0=mybir.AluOpType.add,
                        op1=mybir.AluOpType.add,
                    )

                for r in range(2):  # batch within the pair
                    tok0 = r * T
                    # ---- S^T = K @ Q^T per head ----
                    # per-head psum [128, 384]: kc0 x q[0:256] at 0:256,
                    # kc1 x q1 at 256:384. Heads 2j/2j+1 sit on array row
                    # groups 0:64 / 64:128 -> their MMs and LDWs overlap.
                    p_sb = pp.tile([128, H, 384], BF16, name=f"p_sb{r}")
                    for j in range(3):
                        ps_h = [
                            ps_s.tile([128, 384], F32, tag="s", name=f"ps_h{m}")
                            for m in range(2)
                        ]
                        for kc in range(2):
                            for m in range(2):
                                h = 2 * j + m
                                po = m * 64
                                kT = qk_sb[po : po + 64, 3 + j, tok0 : tok0 + T]
                                qT = qk_sb[po : po + 64, h // 2, tok0 : tok0 + T]
                                if kc == 0:
                                    nc.tensor.matmul(
                                        ps_h[m][:, 0:256],
                                        lhsT=kT[:, 0:128],
                                        rhs=qT,
                                        start=True,
                                        stop=True,
                                    )
                                else:
                                    nc.tensor.matmul(
                                        ps_h[m][:, 256:384],
                                        lhsT=kT[:, 128:256],
                                        rhs=qT[:, 128:256],
                                        start=True,
                                        stop=True,
                                    )
                        for m in range(2):
                            h = 2 * j + m
                            nc.scalar.activation(
                                out=p_sb[:, h, :],
                                in_=ps_h[m],
                                func=mybir.ActivationFunctionType.Exp,
                                scale=SCALE,
                            )
                            # causal mask: zero future positions in the two
                            # diagonal blocks (cols 0:128 and 256:384)
                            dv = bass.AP(
                                tensor=p_sb.tensor,
                                offset=p_sb.offset + h * 384,
                                ap=[list(p_sb.ap[0]), [256, 2], [1, 128]],
                            )
                            nc.gpsimd.tensor_tensor(
                                out=dv,
                                in0=dv,
                                in1=bass.AP(
                                    tensor=tri01.tensor,
                                    offset=tri01.offset,
                                    ap=[list(tri01.ap[0]), [0, 2], [1, 128]],
                                ),
                                op=mybir.AluOpType.mult,
                            )

                    # ---- PV + normalize + transpose ----
                    oT_sb = otp.tile([128, 3, T], BF16, name=f"oT_sb{r}")
                    for qc in range(2):
                        ps_pv = ps_o.tile([128, H * (DH + 1)], F32, tag="pv")
                        for h in range(H):
                            pcol = h * (DH + 1)
                            nc.tensor.matmul(
                                ps_pv[:, pcol : pcol + DH + 1],
                                lhsT=p_sb[:, h, qc * 128 : qc * 128 + 128],
                                rhs=v_sb[:, 2 * r, h, :],
                                start=True,
                                stop=(qc == 0),
                            )
                            if qc == 1:
                                nc.tensor.matmul(
                                    ps_pv[:, pcol : pcol + DH + 1],
                                    lhsT=p_sb[:, h, 256:384],
                                    rhs=v_sb[:, 2 * r + 1, h, :],
                                    start=False,
                                    stop=True,
                                )
                        rc = op_pool.tile([128, H], F32, tag="rc")
                        nc.vector.reciprocal(
                            rc,
                            bass.AP(
                                tensor=ps_pv.tensor,
                                offset=ps_pv.offset + DH,
                                ap=[list(ps_pv.ap[0]), [DH + 1, H]],
                            ),
                        )
                        o_sb = op_pool.tile([128, H, DH], BF16, tag="o")
                        nc.vector.tensor_tensor(
                            out=o_sb,
                            in0=ps_pv.rearrange("p (h d) -> p h d", h=H)[:, :, 0:DH],
                            in1=rc.to_broadcast((128, H, DH)),
                            op=mybir.AluOpType.mult,
                        )
                        ps_tr = ps_o.tile([128, 3, 128], BF16, tag="tr", bufs=1)
                        for ci in range(3):
                            nc.tensor.transpose(
                                ps_tr[:, ci, :],
                                o_sb.rearrange("p h d -> p (h d)")[
                                    :, ci * 128 : (ci + 1) * 128
                                ],
                                ident,
                            )
                        nc.scalar.activation(
                            out=bass.AP(
                                tensor=oT_sb.tensor,
                                offset=oT_sb.offset + qc * 128,
                                ap=[list(oT_sb.ap[0]), [T, 3], [1, 128]],
                            ),
                            in_=ps_tr,
                            func=mybir.ActivationFunctionType.Copy,
                        )

                    # ---- projection + bias + store ----
                    for qc in range(2):
                        ps_y = ps_gen.tile([128, C], F32, tag="gen")
                        for ci in range(3):
                            nc.tensor.matmul(
                                ps_y,
                                lhsT=oT_sb[:, ci, qc * 128 : (qc + 1) * 128],
                                rhs=wp_sb[:, ci, :],
                                start=(ci == 0),
                                stop=(ci == 2),
                            )
                        y_sb = yp.tile([128, C], F32)
                        nc.vector.scalar_tensor_tensor(
                            out=y_sb,
                            in0=ps_y,
                            scalar=0.0,
                            in1=bp_bc,
                            op0=mybir.AluOpType.add,
                            op1=mybir.AluOpType.add,
                        )
                        row0 = (2 * bb + r) * T + qc * 128
                        nc.sync.dma_start(out=out[row0 : row0 + 128, :], in_=y_sb)
    nc.compile()
    return nc


def kernel(x, W_qkv, b_qkv, W_proj, b_proj):
    x = np.asarray(x, dtype=np.float32)
    W_qkv = np.asarray(W_qkv, dtype=np.float32)
    b_qkv = np.asarray(b_qkv, dtype=np.float32)
    W_proj = np.asarray(W_proj, dtype=np.float32)
    b_proj = np.asarray(b_proj, dtype=np.float32)

    if "nc" not in _CACHE:
        _CACHE["nc"] = build_kernel()
    nc = _CACHE["nc"]

    wqkvT = np.ascontiguousarray(W_qkv.T).astype(ml_dtypes.bfloat16)
    wprojT = np.ascontiguousarray(W_proj.T).astype(ml_dtypes.bfloat16)
    in_maps = []
    for i in range(N_CORES):
        xs = x[i * BPC : (i + 1) * BPC].reshape(NTOK, C)
        xTl = np.ascontiguousarray(xs.T).astype(ml_dtypes.bfloat16)
        in_maps.append(
            {
                "xT": xTl,
                "wqkvT": wqkvT,
                "wprojT": wprojT,
                "bqkv": b_qkv,
                "bproj": b_proj,
            }
        )
    res = run_bass_kernel_spmd(nc, in_maps, core_ids=list(range(N_CORES)))
    outs = [res.results[i]["out"].reshape(BPC, T, C) for i in range(N_CORES)]
    return np.concatenate(outs, axis=0).astype(np.float32)


# revision 23
# speedup vs baseline: 1.4710x; 1.1426x over previous
"""Trainium2 Bass kernel for causal multi-head attention block.

B=128, T=256, C=384, H=6, Dh=64. Data-parallel over batch: 16 batches per
core on 8 NeuronCores. Weights replicated; no collectives.

Per-core dataflow (all feature-major until the PV output):
  XT [384, 4096] bf16 (host pre-transposed), processed 2 batches (512 tok)
  per iteration:
  QKV^T: Q^T/K^T feature-major [768, 512], V token-major [512, 384+1]
  S^T[k,q] = K Q^T per head/batch (causal-skipped)
  P = exp(S^T * 1/8) on ACT (no max subtraction; |S*scale| small),
  causal mask applied multiplicatively on GPSIMD
  O[q, 65] = P^T.T @ V_aug  (P-stationary) -> rowsum in col 64
  normalize per-partition (q), PE-transpose O -> O^T, proj, +bias, DMA out.
"""

import sys

sys.path.insert(0, "/opt/trn_rl_repo")

import numpy as np
import ml_dtypes

import concourse.bass as bass
import concourse.mybir as mybir
import concourse.tile as tile
from concourse import bacc
from concourse.bass_utils import run_bass_kernel_spmd
from concourse.masks import make_identity

BF16 = mybir.dt.bfloat16
F32 = mybir.dt.float32

N_CORES = 8
B_FULL, T, C = 128, 256, 384
H, DH = 6, 64
BPC = B_FULL // N_CORES  # 16 batches per core
NTOK = BPC * T  # 4096 tokens per core
SCALE = 1.0 / 8.0  # 1/sqrt(64)

_CACHE = {}


def build_kernel():
    nc = bacc.Bacc()
    xT = nc.declare_dram_parameter("xT", [C, NTOK], BF16, isOutput=False)
    wqkvT = nc.declare_dram_parameter("wqkvT", [C, 3 * C], BF16, isOutput=False)
    wprojT = nc.declare_dram_parameter("wprojT", [C, C], BF16, isOutput=False)
    bqkv = nc.declare_dram_parameter("bqkv", [3 * C], F32, isOutput=False)
    bproj = nc.declare_dram_parameter("bproj", [C], F32, isOutput=False)
    out = nc.declare_dram_parameter("out", [NTOK, C], F32, isOutput=True)

    T2 = 4 * T  # tokens per iteration (4 batches)

    with tile.TileContext(nc) as tc:
        with (
            tc.tile_pool(name="consts", bufs=1) as consts,
            tc.tile_pool(name="xbp", bufs=3) as xbp,
            tc.tile_pool(name="qkp", bufs=2) as qkp,
            tc.tile_pool(name="vp", bufs=2) as vp,
            tc.tile_pool(name="pp", bufs=3) as pp,
            tc.tile_pool(name="op", bufs=3) as op_pool,
            tc.tile_pool(name="otp", bufs=3) as otp,
            tc.tile_pool(name="yp", bufs=3) as yp,
            tc.tile_pool(name="ps_gen", bufs=2, space="PSUM") as ps_gen,
            tc.tile_pool(name="ps_s", bufs=2, space="PSUM") as ps_s,
            tc.tile_pool(name="ps_o", bufs=2, space="PSUM") as ps_o,
        ):
            # ---- constants ----
            w_sb = consts.tile([128, 3, 3 * C], BF16)  # wqkvT chunks
            nc.sync.dma_start(
                out=w_sb, in_=wqkvT[:].rearrange("(a p) c -> p a c", p=128)
            )
            wp_sb = consts.tile([128, 3, C], BF16)
            nc.sync.dma_start(
                out=wp_sb, in_=wprojT[:].rearrange("(a p) c -> p a c", p=128)
            )
            bqk_sb = consts.tile([128, 6], F32)  # per-partition bias cols
            nc.sync.dma_start(
                out=bqk_sb,
                in_=bass.AP(tensor=bqkv, offset=0, ap=[[1, 128], [128, 6]]),
            )
            bv_bc = consts.tile([128, C], F32)  # V bias broadcast over partitions
            nc.sync.dma_start(
                out=bv_bc,
                in_=bass.AP(tensor=bqkv, offset=2 * C, ap=[[0, 128], [1, C]]),
            )
            bp_bc = consts.tile([128, C], F32)
            nc.sync.dma_start(
                out=bp_bc,
                in_=bass.AP(tensor=bproj, offset=0, ap=[[0, 128], [1, C]]),
            )
            # prime DVE's observed DMA ticks so per-batch evac ops carry only
            # the PE wait (the DVE TT ISA struct has a single wait slot).
            for i, cst in enumerate((bqk_sb, bv_bc, bp_bc)):
                scratch = consts.tile([128, 1], F32, tag=f"scr{i}")
                nc.vector.tensor_copy(scratch, cst[:, 0:1])

            ident = consts.tile([128, 128], BF16)
            make_identity(nc, ident)
            # tri01[k, q] = 1 where k <= q else 0 (multiplicative causal mask)
            tri01 = consts.tile([128, 128], BF16)
            nc.gpsimd.memset(tri01, 1.0)
            nc.gpsimd.affine_select(
                out=tri01,
                in_=tri01,
                compare_op=mybir.AluOpType.is_ge,
                fill=0.0,
                base=0,
                pattern=[[1, 128]],
                channel_multiplier=-1,
            )

            for bb in range(BPC // 4):
                # ---- load x^T slice for this batch quad ----
                xb = xbp.tile([128, 3, T2], BF16)
                nc.sync.dma_start(
                    out=xb,
                    in_=bass.AP(
                        tensor=xT,
                        offset=bb * T2,
                        ap=[[NTOK, 128], [128 * NTOK, 3], [1, T2]],
                    ),
                )

                # ---- QKV projection (2 batches at once) ----
                qk_sb = qkp.tile([128, 6, T2], BF16)
                for f in range(6):
                    ps_f = ps_gen.tile([128, 2, 512], F32, tag="gen")
                    for half in range(2):
                        for ci in range(3):
                            nc.tensor.matmul(
                                ps_f[:, half, :],
                                lhsT=w_sb[:, ci, f * 128 : (f + 1) * 128],
                                rhs=xb[:, ci, half * 512 : (half + 1) * 512],
                                start=(ci == 0),
                                stop=(ci == 2),
                            )
                    nc.vector.tensor_tensor(
                        out=qk_sb[:, f, :],
                        in0=ps_f.rearrange("p a b -> p (a b)"),
                        in1=bqk_sb[:, f : f + 1].to_broadcast((128, T2)),
                        op=mybir.AluOpType.add,
                    )
                # V token-major with ones column: [128, 4 tok-chunks, 6, 65]
                v_sb = vp.tile([128, 8, H, DH + 1], BF16)
                nc.vector.memset(v_sb[:, :, :, DH : DH + 1], 1.0)
                for tc_i in range(8):
                    ps_v = ps_gen.tile([128, C], F32, tag="gen")
                    for ci in range(3):
                        nc.tensor.matmul(
                            ps_v,
                            lhsT=xb[:, ci, tc_i * 128 : (tc_i + 1) * 128],
                            rhs=w_sb[:, ci, 2 * C : 3 * C],
                            start=(ci == 0),
                            stop=(ci == 2),
                        )
                    nc.vector.scalar_tensor_tensor(
                        out=v_sb[:, tc_i, :, 0:DH],
                        in0=ps_v.rearrange("p (h d) -> p h d", h=H),
                        scalar=0.0,
                        in1=bv_bc.rearrange("p (h d) -> p h d", h=H),
                        op0=mybir.AluOpType.add,
                        op1=mybir.AluOpType.add,
                    )

                p_sbs = {}
                for r in range(4):  # batch within the quad
                    tok0 = r * T
                    # ---- S^T = K @ Q^T per head ----
                    # per-head psum [128, 384]: kc0 x q[0:256] at 0:256,
                    # kc1 x q1 at 256:384. Heads 2j/2j+1 sit on array row
                    # groups 0:64 / 64:128 -> their MMs and LDWs overlap.
                    p_sb = pp.tile([128, H, 384], BF16, name=f"p_sb{r}", tag=f"p{r%2}")
                    p_sbs[r] = p_sb
                    for j in range(3):
                        ps_h = [
                            ps_s.tile([128, 384], F32, tag="s", name=f"ps_h{m}")
                            for m in range(2)
                        ]
                        for kc in range(2):
                            for m in range(2):
                                h = 2 * j + m
                                po = m * 64
                                kT = qk_sb[po : po + 64, 3 + j, tok0 : tok0 + T]
                                qT = qk_sb[po : po + 64, h // 2, tok0 : tok0 + T]
                                if kc == 0:
                                    nc.tensor.matmul(
                                        ps_h[m][:, 0:256],
                                        lhsT=kT[:, 0:128],
                                        rhs=qT,
                                        start=True,
                                        stop=True,
                                    )
                                else:
                                    nc.tensor.matmul(
                                        ps_h[m][:, 256:384],
                                        lhsT=kT[:, 128:256],
                                        rhs=qT[:, 128:256],
                                        start=True,
                                        stop=True,
                                    )
                        for m in range(2):
                            h = 2 * j + m
                            nc.scalar.activation(
                                out=p_sb[:, h, :],
                                in_=ps_h[m],
                                func=mybir.ActivationFunctionType.Exp,
                                scale=SCALE,
                            )
                            # causal mask: zero future positions in the two
                            # diagonal blocks (cols 0:128 and 256:384)
                            dv = bass.AP(
                                tensor=p_sb.tensor,
                                offset=p_sb.offset + h * 384,
                                ap=[list(p_sb.ap[0]), [256, 2], [1, 128]],
                            )
                            nc.gpsimd.tensor_tensor(
                                out=dv,
                                in0=dv,
                                in1=bass.AP(
                                    tensor=tri01.tensor,
                                    offset=tri01.offset,
                                    ap=[list(tri01.ap[0]), [0, 2], [1, 128]],
                                ),
                                op=mybir.AluOpType.mult,
                            )

                oT_sbs = {}
                for r in range(4):
                    p_sb = p_sbs[r]
                    # ---- PV + normalize + transpose ----
                    oT_sb = otp.tile([128, 3, T], BF16, name=f"oT_sb{r}", tag=f"ot{r%2}")
                    oT_sbs[r] = oT_sb
                    for qc in range(2):
                        ps_pv = ps_o.tile([128, H * (DH + 1)], F32, tag="pv")
                        for h in range(H):
                            pcol = h * (DH + 1)
                            nc.tensor.matmul(
                                ps_pv[:, pcol : pcol + DH + 1],
                                lhsT=p_sb[:, h, qc * 128 : qc * 128 + 128],
                                rhs=v_sb[:, 2 * r, h, :],
                                start=True,
                                stop=(qc == 0),
                            )
                            if qc == 1:
                                nc.tensor.matmul(
                                    ps_pv[:, pcol : pcol + DH + 1],
                                    lhsT=p_sb[:, h, 256:384],
                                    rhs=v_sb[:, 2 * r + 1, h, :],
                                    start=False,
                                    stop=True,
                                )
                        rc = op_pool.tile([128, H], F32, tag="rc")
                        nc.vector.reciprocal(
                            rc,
                            bass.AP(
                                tensor=ps_pv.tensor,
                                offset=ps_pv.offset + DH,
                                ap=[list(ps_pv.ap[0]), [DH + 1, H]],
                            ),
                        )
                        o_sb = op_pool.tile([128, H, DH], BF16, tag="o")
                        nc.vector.tensor_tensor(
                            out=o_sb,
                            in0=ps_pv.rearrange("p (h d) -> p h d", h=H)[:, :, 0:DH],
                            in1=rc.to_broadcast((128, H, DH)),
                            op=mybir.AluOpType.mult,
                        )
                        ps_tr = ps_o.tile([128, 3, 128], BF16, tag="pv")
                        for ci in range(3):
                            nc.tensor.transpose(
                                ps_tr[:, ci, :],
                                o_sb.rearrange("p h d -> p (h d)")[
                                    :, ci * 128 : (ci + 1) * 128
                                ],
                                ident,
                            )
                        nc.scalar.activation(
                            out=bass.AP(
                                tensor=oT_sb.tensor,
                                offset=oT_sb.offset + qc * 128,
                                ap=[list(oT_sb.ap[0]), [T, 3], [1, 128]],
                            ),
                            in_=ps_tr,
                            func=mybir.ActivationFunctionType.Copy,
                        )

                for r in range(4):
                    oT_sb = oT_sbs[r]
                    # ---- projection + bias + store ----
                    for qc in range(2):
                        ps_y = ps_gen.tile([128, C], F32, tag="gen")
                        for ci in range(3):
                            nc.tensor.matmul(
                                ps_y,
                                lhsT=oT_sb[:, ci, qc * 128 : (qc + 1) * 128],
                                rhs=wp_sb[:, ci, :],
                                start=(ci == 0),
                                stop=(ci == 2),
                            )
                        y_sb = yp.tile([128, C], F32)
                        nc.vector.scalar_tensor_tensor(
                            out=y_sb,
                            in0=ps_y,
                            scalar=0.0,
                            in1=bp_bc,
                            op0=mybir.AluOpType.add,
                            op1=mybir.AluOpType.add,
                        )
                        row0 = (4 * bb + r) * T + qc * 128
                        nc.sync.dma_start(out=out[row0 : row0 + 128, :], in_=y_sb)
    nc.compile()
    return nc


def kernel(x, W_qkv, b_qkv, W_proj, b_proj):
    x = np.asarray(x, dtype=np.float32)
    W_qkv = np.asarray(W_qkv, dtype=np.float32)
    b_qkv = np.asarray(b_qkv, dtype=np.float32)
    W_proj = np.asarray(W_proj, dtype=np.float32)
    b_proj = np.asarray(b_proj, dtype=np.float32)

    if "nc" not in _CACHE:
        _CACHE["nc"] = build_kernel()
    nc = _CACHE["nc"]

    wqkvT = np.ascontiguousarray(W_qkv.T).astype(ml_dtypes.bfloat16)
    wprojT = np.ascontiguousarray(W_proj.T).astype(ml_dtypes.bfloat16)
    in_maps = []
    for i in range(N_CORES):
        xs = x[i * BPC : (i + 1) * BPC].reshape(NTOK, C)
        xTl = np.ascontiguousarray(xs.T).astype(ml_dtypes.bfloat16)
        in_maps.append(
            {
                "xT": xTl,
                "wqkvT": wqkvT,
                "wprojT": wprojT,
                "bqkv": b_qkv,
                "bproj": b_proj,
            }
        )
    res = run_bass_kernel_spmd(nc, in_maps, core_ids=list(range(N_CORES)))
    outs = [res.results[i]["out"].reshape(BPC, T, C) for i in range(N_CORES)]
    return np.concatenate(outs, axis=0).astype(np.float32)
